# revision 10
# baseline (speedup 1.0000x reference)
"""AdditiveAttention kernel for 8 TRN2 NeuronCores.

Problem shapes (hardcoded): q [4,512,512], k [4,512,512], v [4,512,256],
Wq [256,512], Wk [256,512], wv [256].

reference:
    qh = q @ Wq.T            [B, QN, H]
    kh = k @ Wk.T            [B, KVN, H]
    scores[b,q,k] = sum_h wv[h] * tanh(qh[b,q,h] + kh[b,k,h])
    attn = softmax(scores, axis=-1)
    out = attn @ v
    returns (out, attn)

Sharding: core c handles batch b = c//2, query rows (c%2)*256:(c%2+1)*256.
k, v, Wq, Wk, wv replicated per batch. No collectives.

Per-core kernel strategy:
  - qh, kh computed on-chip in [h(part), seq(free)] layout (PE transposes
    of q/k/W via identity, then f32 matmuls).
  - energy S = qh[:,q] + kh  via DVE tensor_scalar_add (per-partition scalar),
    batched QB queries per buffer; tanh in one big ACT instruction (in-place).
  - scores row for query q via PE matvec with a sliding-window one-hot
    stationary: wv_pad [128, 255] zeros with wv at col 127; lhsT window
    [:, 127-j:255-j] puts wv in column j so the matvec writes PSUM
    partition j (all other partitions accumulate zero). 256 accumulating
    matmuls build a full [128 q, 512 k] scores tile in one PSUM bank.
    float32r (1 cycle/row) for speed.
  - softmax over free dim: ACT Exp with accum_out (row sums), DVE
    reciprocal + tensor_scalar_mul.
  - out = attn @ v via PE transpose of attn blocks + f32 matmuls.
"""

import os

import numpy as np

B, QN, KVN = 4, 512, 512
QD, KD, H, VD = 512, 512, 256, 256
NCORES = 8
QSH = QN // 2  # 256 query rows per core
P = 128
QB = 8  # queries per tanh batch

_last_results = None


def _ensure_ntff_hook():
    """Register the NTFF profile hook so trace=True works under axon.

    The agent image's antenv package lacks axon_hooks, so trn_boot's
    silent-degrade path left concourse without a hook. Inject an
    in-memory module and install the ctypes hook from trn_agent_boot.
    """
    import sys
    import types

    try:
        import antenv.axon_hooks  # noqa: F401

        return
    except ImportError:
        pass
    try:
        import antenv
    except ImportError:
        return
    mod = types.ModuleType("antenv.axon_hooks")
    mod._hook = None

    def set_axon_ntff_profile_hook(hook):
        mod._hook = hook

    def get_axon_ntff_profile_hook():
        return mod._hook

    mod.set_axon_ntff_profile_hook = set_axon_ntff_profile_hook
    mod.get_axon_ntff_profile_hook = get_axon_ntff_profile_hook
    sys.modules["antenv.axon_hooks"] = mod
    antenv.axon_hooks = mod
    try:
        from trn_agent_boot.trn_boot import _ntff_profile_via_ctypes

        hook = _ntff_profile_via_ctypes("/opt/axon/libaxon_pjrt.so")
        if hook is not None:
            mod._hook = hook
    except Exception:
        pass


def _build_bass():
    import concourse.bass as bass  # noqa: F401
    import concourse.mybir as mybir
    import concourse.tile as tile
    from concourse import bacc
    from concourse.masks import make_identity

    f32 = mybir.dt.float32
    bf16 = mybir.dt.bfloat16
    AF = mybir.ActivationFunctionType

    nc = bacc.Bacc()

    q_d = nc.declare_dram_parameter("q", [QSH, QD], f32, isOutput=False)
    k_d = nc.declare_dram_parameter("k", [KVN, KD], f32, isOutput=False)
    v_d = nc.declare_dram_parameter("v", [KVN, VD], f32, isOutput=False)
    wq_d = nc.declare_dram_parameter("Wq", [H, QD], f32, isOutput=False)
    wk_d = nc.declare_dram_parameter("Wk", [H, KD], f32, isOutput=False)
    wv_d = nc.declare_dram_parameter("wv", [H], f32, isOutput=False)
    oy_d = nc.declare_dram_parameter("out_y", [QSH, VD], f32, isOutput=True)
    oa_d = nc.declare_dram_parameter("out_a", [QSH, KVN], f32, isOutput=True)

    DC = QD // P  # 4 contraction chunks
    HC = H // P  # 2 h chunks
    KC = KVN // P  # 4 kv chunks
    QT = QSH // P  # 2 query tiles per core

    with tile.TileContext(nc) as tc:
        with (
            tc.tile_pool(name="const", bufs=1) as const,
            tc.tile_pool(name="work", bufs=2) as work,
            tc.tile_pool(name="spool", bufs=2) as spool,
            tc.tile_pool(name="pre", bufs=2) as pre,
            tc.tile_pool(name="psum", bufs=1, space="PSUM") as psum,
        ):
            # ---- constants ----
            ident = const.tile([P, P], f32)
            make_identity(nc, ident)

            wv_pad = const.tile([P, HC, 2 * P - 1], bf16)
            nc.gpsimd.memset(wv_pad, 0.0)
            wv_f32 = const.tile([P, HC], f32)
            for c in range(HC):
                nc.sync.dma_start(
                    out=wv_f32[:, c : c + 1], in_=wv_d[c * P : (c + 1) * P]
                )
                nc.vector.tensor_copy(wv_pad[:, c, P - 1 : P], wv_f32[:, c : c + 1])

            # ---- load v (natural [k, vd] layout, what the output matmul needs)
            v_sb = const.tile([P, KC, VD], f32)
            for kc in range(KC):
                nc.sync.dma_start(out=v_sb[:, kc, :], in_=v_d[kc * P : (kc + 1) * P, :])

            # ---- transposed loads: xT [d(part), seq] via PE transpose ----
            qT = const.tile([P, DC, QSH], f32)
            kT = const.tile([P, DC, KVN], f32)
            wqT = const.tile([P, DC, H], f32)
            wkT = const.tile([P, DC, H], f32)

            def load_transposed(dram, rows, dst, dst_col0):
                # dram block rows [rows, 512] -> dst[:, dc, dst_col0:dst_col0+rows]
                nat = pre.tile([P, QD], f32, tag="nat")
                nc.sync.dma_start(out=nat, in_=dram)
                for dc in range(DC):
                    pt = psum.tile([P, P], f32, tag="attn_t", bufs=2)
                    nc.tensor.transpose(pt, nat[:, dc * P : (dc + 1) * P], ident)
                    nc.any.tensor_copy(
                        out=dst[:, dc, dst_col0 : dst_col0 + rows], in_=pt
                    )

            for i in range(QT):
                load_transposed(q_d[i * P : (i + 1) * P, :], P, qT, i * P)
            for i in range(KC):
                load_transposed(k_d[i * P : (i + 1) * P, :], P, kT, i * P)
            for i in range(HC):
                load_transposed(wq_d[i * P : (i + 1) * P, :], P, wqT, i * P)
            for i in range(HC):
                load_transposed(wk_d[i * P : (i + 1) * P, :], P, wkT, i * P)

            # ---- qh [h, q], kh [h, k] ----
            qh = const.tile([P, HC, QSH], f32)
            kh = const.tile([P, HC, KVN], f32)
            for c in range(HC):
                pq = psum.tile([P, QSH], f32, tag="sc", bufs=2)
                for dc in range(DC):
                    nc.tensor.matmul(
                        pq,
                        wqT[:, dc, c * P : (c + 1) * P],
                        qT[:, dc, :],
                        start=(dc == 0),
                        stop=(dc == DC - 1),
                    )
                nc.any.tensor_copy(out=qh[:, c, :], in_=pq)
            for c in range(HC):
                pk = psum.tile([P, KVN], f32, tag="sc", bufs=2)
                for dc in range(DC):
                    nc.tensor.matmul(
                        pk,
                        wkT[:, dc, c * P : (c + 1) * P],
                        kT[:, dc, :],
                        start=(dc == 0),
                        stop=(dc == DC - 1),
                    )
                nc.any.tensor_copy(out=kh[:, c, :], in_=pk)

            # ---- main loop over query tiles ----
            NB = P // QB  # blocks per query tile
            for qt in range(QT):
                psc = psum.tile([P, KVN], f32, tag="sc", bufs=2)
                for qb in range(NB):
                    S = spool.tile([P, HC, QB, KVN], f32, tag="S")
                    E = spool.tile([P, HC, QB, KVN], bf16, tag="E")
                    for j in range(QB):
                        qg = qt * P + qb * QB + j
                        for c in range(HC):
                            nc.vector.tensor_scalar_add(
                                S[:, c, j, :], kh[:, c, :], qh[:, c, qg : qg + 1]
                            )
                    for c in range(HC):
                        nc.scalar.activation(E[:, c], S[:, c], AF.Tanh)
                    for j in range(QB):
                        ql = qb * QB + j
                        for c in range(HC):
                            nc.tensor.matmul(
                                psc,
                                wv_pad[:, c, P - 1 - ql : 2 * P - 1 - ql],
                                E[:, c, j, :],
                                start=(qb == 0 and j == 0 and c == 0),
                                stop=(qb == NB - 1 and j == QB - 1 and c == HC - 1),
                            )

                # softmax over k (free dim); no max subtraction needed
                # (|scores| <= ||wv||_1 ~ 13, exp stays in f32 range)
                probs = work.tile([P, KVN], f32, tag="probs")
                sums = work.tile([P, 1], f32, tag="sums")
                nc.scalar.activation(probs, psc, AF.Exp, accum_out=sums)
                rinv = work.tile([P, 1], f32, tag="rinv")
                nc.vector.reciprocal(rinv, sums)
                attn = work.tile([P, KVN], f32, tag="attn")
                nc.vector.tensor_scalar_mul(attn, probs, rinv)
                nc.sync.dma_start(out=oa_d[qt * P : (qt + 1) * P, :], in_=attn)

                # out = attn @ v : transpose attn blocks, accumulate over k
                po = psum.tile([P, VD], f32, tag="po", bufs=2)
                for kc in range(KC):
                    ptr = psum.tile([P, P], f32, tag="attn_t", bufs=2)
                    nc.tensor.transpose(
                        ptr, attn[:, kc * P : (kc + 1) * P], ident
                    )
                    attnT = work.tile([P, P], f32, tag="attnT")
                    nc.vector.tensor_copy(attnT, ptr)
                    nc.tensor.matmul(
                        po,
                        attnT,
                        v_sb[:, kc, :],
                        start=(kc == 0),
                        stop=(kc == KC - 1),
                    )
                out_sb = work.tile([P, VD], f32, tag="out_sb")
                nc.vector.tensor_copy(out_sb, po)
                nc.sync.dma_start(out=oy_d[qt * P : (qt + 1) * P, :], in_=out_sb)

    nc.finalize()
    return nc


def kernel(q, k, v, Wq, Wk, wv):
    global _last_results
    _ensure_ntff_hook()
    from concourse.bass_utils import run_bass_kernel_spmd

    q = np.ascontiguousarray(np.asarray(q, dtype=np.float32))
    k = np.ascontiguousarray(np.asarray(k, dtype=np.float32))
    v = np.ascontiguousarray(np.asarray(v, dtype=np.float32))
    Wq = np.ascontiguousarray(np.asarray(Wq, dtype=np.float32))
    Wk = np.ascontiguousarray(np.asarray(Wk, dtype=np.float32))
    wv = np.ascontiguousarray(np.asarray(wv, dtype=np.float32))

    nc = _build_bass()

    in_maps = []
    for c in range(NCORES):
        b, half = c // 2, c % 2
        in_maps.append(
            {
                "q": np.ascontiguousarray(q[b, half * QSH : (half + 1) * QSH]),
                "k": k[b],
                "v": v[b],
                "Wq": Wq,
                "Wk": Wk,
                "wv": wv,
            }
        )

    trace = os.environ.get("KERNEL_TRACE", "0") == "1"
    res = run_bass_kernel_spmd(
        nc, in_maps, core_ids=list(range(NCORES)), trace=trace
    )
    _last_results = res

    output = np.empty((B, QN, VD), dtype=np.float32)
    attention = np.empty((B, QN, KVN), dtype=np.float32)
    for c in range(NCORES):
        b, half = c // 2, c % 2
        output[b, half * QSH : (half + 1) * QSH] = res.results[c]["out_y"]
        attention[b, half * QSH : (half + 1) * QSH] = res.results[c]["out_a"]
    return output, attention


# revision 12
# speedup vs baseline: 1.0773x; 1.0773x over previous
"""AdditiveAttention kernel for 8 TRN2 NeuronCores.

Problem shapes (hardcoded): q [4,512,512], k [4,512,512], v [4,512,256],
Wq [256,512], Wk [256,512], wv [256].

reference:
    qh = q @ Wq.T            [B, QN, H]
    kh = k @ Wk.T            [B, KVN, H]
    scores[b,q,k] = sum_h wv[h] * tanh(qh[b,q,h] + kh[b,k,h])
    attn = softmax(scores, axis=-1)
    out = attn @ v
    returns (out, attn)

Sharding: core c handles batch b = c//2, query rows (c%2)*256:(c%2+1)*256.
k, v, Wq, Wk, wv replicated per batch. No collectives.

Per-core kernel strategy (ACT/tanh-roofline bound, ~33.5M tanh elems/core):
  - qh, kh computed on-chip in [h(part), seq(free)] layout (PE transposes
    of q/k/W via identity into bf16, then bf16 matmuls).
  - energy S = qh[:,q] + kh via DVE tensor_scalar_add (per-partition
    scalar, bf16 in/out -> 4x DVE mode), batched QB queries per buffer;
    tanh in one big ACT instruction per h-chunk (bf16 -> bf16).
  - scores row for query q via PE matvec with a sliding-window one-hot
    stationary: wv_pad [128, 2, 255] zeros with wv chunk at col 127; lhsT
    window [:, c, 127-j:255-j] puts wv in column j so the matvec writes
    PSUM partition j (other partitions accumulate zero). 256 accumulating
    bf16 matmuls build a full [128 q, 512 k] scores tile in one PSUM bank.
  - softmax over free dim: ACT Exp (PSUM src) with accum_out row sums,
    DVE reciprocal + tensor_scalar_mul -> attn f32 -> DMA out.
  - out = attn @ v: PE transpose of attn blocks -> bf16 attnT, bf16 v.
"""

import os

import numpy as np

B, QN, KVN = 4, 512, 512
QD, KD, H, VD = 512, 512, 256, 256
NCORES = 8
QSH = QN // 2  # 256 query rows per core
P = 128
QB = 16  # queries per tanh batch

_last_results = None


def _ensure_ntff_hook():
    """Register the NTFF profile hook so trace=True works under axon.

    The agent image's antenv package lacks axon_hooks, so trn_boot's
    silent-degrade path left concourse without a hook. Inject an
    in-memory module and install the ctypes hook from trn_agent_boot.
    """
    import sys
    import types

    try:
        import antenv.axon_hooks  # noqa: F401

        return
    except ImportError:
        pass
    try:
        import antenv
    except ImportError:
        return
    mod = types.ModuleType("antenv.axon_hooks")
    mod._hook = None

    def set_axon_ntff_profile_hook(hook):
        mod._hook = hook

    def get_axon_ntff_profile_hook():
        return mod._hook

    mod.set_axon_ntff_profile_hook = set_axon_ntff_profile_hook
    mod.get_axon_ntff_profile_hook = get_axon_ntff_profile_hook
    sys.modules["antenv.axon_hooks"] = mod
    antenv.axon_hooks = mod
    try:
        from trn_agent_boot.trn_boot import _ntff_profile_via_ctypes

        hook = _ntff_profile_via_ctypes("/opt/axon/libaxon_pjrt.so")
        if hook is not None:
            mod._hook = hook
    except Exception:
        pass


def _build_bass():
    import concourse.bass as bass  # noqa: F401
    import concourse.mybir as mybir
    import concourse.tile as tile
    from concourse import bacc
    from concourse.masks import make_identity

    f32 = mybir.dt.float32
    bf16 = mybir.dt.bfloat16
    AF = mybir.ActivationFunctionType

    nc = bacc.Bacc()

    q_d = nc.declare_dram_parameter("q", [QSH, QD], f32, isOutput=False)
    k_d = nc.declare_dram_parameter("k", [KVN, KD], f32, isOutput=False)
    v_d = nc.declare_dram_parameter("v", [KVN, VD], f32, isOutput=False)
    wq_d = nc.declare_dram_parameter("Wq", [H, QD], f32, isOutput=False)
    wk_d = nc.declare_dram_parameter("Wk", [H, KD], f32, isOutput=False)
    wv_d = nc.declare_dram_parameter("wv", [H], f32, isOutput=False)
    oy_d = nc.declare_dram_parameter("out_y", [QSH, VD], f32, isOutput=True)
    oa_d = nc.declare_dram_parameter("out_a", [QSH, KVN], f32, isOutput=True)

    DC = QD // P  # 4 contraction chunks
    HC = H // P  # 2 h chunks
    KC = KVN // P  # 4 kv chunks
    QT = QSH // P  # 2 query tiles per core

    with tile.TileContext(nc) as tc:
        with (
            tc.tile_pool(name="const", bufs=1) as const,
            tc.tile_pool(name="work", bufs=2) as work,
            tc.tile_pool(name="spool", bufs=2) as spool,
            tc.tile_pool(name="pre", bufs=4) as pre,
            tc.tile_pool(name="psum", bufs=1, space="PSUM") as psum,
        ):
            # ---- constants ----
            ident = const.tile([P, P], f32)
            make_identity(nc, ident)

            wv_pad = const.tile([P, HC, 2 * P - 1], bf16)
            nc.vector.memset(wv_pad, 0.0)
            wv_f32 = const.tile([P, HC], f32)
            for c in range(HC):
                nc.sync.dma_start(
                    out=wv_f32[:, c : c + 1], in_=wv_d[c * P : (c + 1) * P]
                )
                nc.vector.tensor_copy(wv_pad[:, c, P - 1 : P], wv_f32[:, c : c + 1])

            # ---- transposed loads: xT [d(part), seq] bf16, via PE transpose
            kT = const.tile([P, DC, KVN], bf16)
            qT = const.tile([P, DC, QSH], bf16)
            wqT = const.tile([P, DC, H], bf16)
            wkT = const.tile([P, DC, H], bf16)

            def load_transposed(dram, dst, dst_col0):
                nat = pre.tile([P, QD], f32, tag="nat")
                nc.sync.dma_start(out=nat, in_=dram)
                for dc in range(DC):
                    pt = psum.tile([P, P], f32, tag="pre_t", bufs=3)
                    nc.tensor.transpose(pt, nat[:, dc * P : (dc + 1) * P], ident)
                    nc.any.tensor_copy(
                        out=dst[:, dc, dst_col0 : dst_col0 + P], in_=pt
                    )

            # order: Wk, k first (kh feeds the first adds), then Wq, q
            for i in range(HC):
                load_transposed(wk_d[i * P : (i + 1) * P, :], wkT, i * P)
            for i in range(KC):
                load_transposed(k_d[i * P : (i + 1) * P, :], kT, i * P)
            for i in range(HC):
                load_transposed(wq_d[i * P : (i + 1) * P, :], wqT, i * P)
            for i in range(QT):
                load_transposed(q_d[i * P : (i + 1) * P, :], qT, i * P)

            # ---- kh [h, k] then qh [h, q], both bf16 ----
            kh = const.tile([P, HC, KVN], bf16)
            qh = const.tile([P, HC, QSH], f32)
            for c in range(HC):
                pk = psum.tile([P, KVN], f32, tag="sc", bufs=2)
                for dc in range(DC):
                    nc.tensor.matmul(
                        pk,
                        wkT[:, dc, c * P : (c + 1) * P],
                        kT[:, dc, :],
                        start=(dc == 0),
                        stop=(dc == DC - 1),
                    )
                nc.any.tensor_copy(out=kh[:, c, :], in_=pk)
            for c in range(HC):
                pq = psum.tile([P, QSH], f32, tag="sc", bufs=2)
                for dc in range(DC):
                    nc.tensor.matmul(
                        pq,
                        wqT[:, dc, c * P : (c + 1) * P],
                        qT[:, dc, :],
                        start=(dc == 0),
                        stop=(dc == DC - 1),
                    )
                nc.any.tensor_copy(out=qh[:, c, :], in_=pq)

            # ---- v in bf16 [k, vd] (for the output matmul) ----
            v_sb = const.tile([P, KC, VD], bf16)
            for kc in range(KC):
                vnat = pre.tile([P, VD], f32, tag="vnat", bufs=2)
                nc.sync.dma_start(out=vnat, in_=v_d[kc * P : (kc + 1) * P, :])
                nc.vector.tensor_copy(v_sb[:, kc, :], vnat)

            # ---- main loop over query tiles ----
            NB = P // QB  # blocks per query tile
            for qt in range(QT):
                psc = psum.tile([P, KVN], f32, tag="sc", bufs=2)
                for qb in range(NB):
                    S = spool.tile([P, HC, QB, KVN], bf16, tag="S")
                    E = spool.tile([P, HC, QB, KVN], bf16, tag="E")
                    for j in range(QB):
                        qg = qt * P + qb * QB + j
                        for c in range(HC):
                            nc.vector.tensor_scalar_add(
                                S[:, c, j, :], kh[:, c, :], qh[:, c, qg : qg + 1]
                            )
                    # split the very last tanh batch so the final matvec
                    # burst (tail latency) is halved
                    last = qt == QT - 1 and qb == NB - 1
                    for c in range(HC):
                        if last:
                            h = QB // 2
                            nc.scalar.activation(
                                E[:, c, :h], S[:, c, :h], AF.Tanh
                            )
                            nc.scalar.activation(
                                E[:, c, h:], S[:, c, h:], AF.Tanh
                            )
                        else:
                            nc.scalar.activation(E[:, c], S[:, c], AF.Tanh)
                    for j in range(QB):
                        ql = qb * QB + j
                        for c in range(HC):
                            nc.tensor.matmul(
                                psc,
                                wv_pad[:, c, P - 1 - ql : 2 * P - 1 - ql],
                                E[:, c, j, :],
                                start=(qb == 0 and j == 0 and c == 0),
                                stop=(qb == NB - 1 and j == QB - 1 and c == HC - 1),
                            )

                # softmax over k (free dim); no max subtraction needed
                # (|scores| <= ||wv||_1 ~ 13, exp stays in f32 range)
                probs = work.tile([P, KVN], f32, tag="probs")
                sums = work.tile([P, 1], f32, tag="sums")
                nc.scalar.activation(probs, psc, AF.Exp, accum_out=sums)
                rinv = work.tile([P, 1], f32, tag="rinv")
                nc.vector.reciprocal(rinv, sums)
                attn = work.tile([P, KVN], f32, tag="attn")
                nc.vector.tensor_scalar_mul(attn, probs, rinv)
                nc.sync.dma_start(out=oa_d[qt * P : (qt + 1) * P, :], in_=attn)

                # out = attn @ v : transpose attn blocks, accumulate over k
                po = psum.tile([P, VD], f32, tag="po", bufs=1)
                for kc in range(KC):
                    ptr = psum.tile([P, P], f32, tag="attn_t", bufs=2)
                    nc.tensor.transpose(
                        ptr, attn[:, kc * P : (kc + 1) * P], ident
                    )
                    attnT = work.tile([P, P], bf16, tag="attnT")
                    nc.vector.tensor_copy(attnT, ptr)
                    nc.tensor.matmul(
                        po,
                        attnT,
                        v_sb[:, kc, :],
                        start=(kc == 0),
                        stop=(kc == KC - 1),
                    )
                out_sb = work.tile([P, VD], f32, tag="out_sb")
                nc.vector.tensor_copy(out_sb, po)
                nc.sync.dma_start(out=oy_d[qt * P : (qt + 1) * P, :], in_=out_sb)

    nc.finalize()
    return nc


def kernel(q, k, v, Wq, Wk, wv):
    global _last_results
    _ensure_ntff_hook()
    from concourse.bass_utils import run_bass_kernel_spmd

    q = np.ascontiguousarray(np.asarray(q, dtype=np.float32))
    k = np.ascontiguousarray(np.asarray(k, dtype=np.float32))
    v = np.ascontiguousarray(np.asarray(v, dtype=np.float32))
    Wq = np.ascontiguousarray(np.asarray(Wq, dtype=np.float32))
    Wk = np.ascontiguousarray(np.asarray(Wk, dtype=np.float32))
    wv = np.ascontiguousarray(np.asarray(wv, dtype=np.float32))

    nc = _build_bass()

    in_maps = []
    for c in range(NCORES):
        b, half = c // 2, c % 2
        in_maps.append(
            {
                "q": np.ascontiguousarray(q[b, half * QSH : (half + 1) * QSH]),
                "k": k[b],
                "v": v[b],
                "Wq": Wq,
                "Wk": Wk,
                "wv": wv,
            }
        )

    trace = os.environ.get("KERNEL_TRACE", "0") == "1"
    res = run_bass_kernel_spmd(
        nc, in_maps, core_ids=list(range(NCORES)), trace=trace
    )
    _last_results = res

    output = np.empty((B, QN, VD), dtype=np.float32)
    attention = np.empty((B, QN, KVN), dtype=np.float32)
    for c in range(NCORES):
        b, half = c // 2, c % 2
        output[b, half * QSH : (half + 1) * QSH] = res.results[c]["out_y"]
        attention[b, half * QSH : (half + 1) * QSH] = res.results[c]["out_a"]
    return output, attention


# revision 14
# speedup vs baseline: 1.1110x; 1.0312x over previous
"""AdditiveAttention kernel for 8 TRN2 NeuronCores.

Problem shapes (hardcoded): q [4,512,512], k [4,512,512], v [4,512,256],
Wq [256,512], Wk [256,512], wv [256].

reference:
    qh = q @ Wq.T            [B, QN, H]
    kh = k @ Wk.T            [B, KVN, H]
    scores[b,q,k] = sum_h wv[h] * tanh(qh[b,q,h] + kh[b,k,h])
    attn = softmax(scores, axis=-1)
    out = attn @ v
    returns (out, attn)

Sharding: core c handles batch b = c//2, query rows (c%2)*256:(c%2+1)*256.
k, v, Wq, Wk, wv replicated per batch. No collectives.

Per-core kernel strategy (ACT/tanh-roofline bound, ~33.5M tanh elems/core):
  - qh, kh computed on-chip in [h(part), seq(free)] layout (PE transposes
    of q/k/W via identity into bf16, then bf16 matmuls).
  - energy S = qh[:,q] + kh via DVE tensor_scalar_add (per-partition
    scalar, bf16 in/out -> 4x DVE mode), batched QB queries per buffer;
    tanh in one big ACT instruction per h-chunk (bf16 -> bf16).
  - scores row for query q via PE matvec with a sliding-window one-hot
    stationary: wv_pad [128, 2, 255] zeros with wv chunk at col 127; lhsT
    window [:, c, 127-j:255-j] puts wv in column j so the matvec writes
    PSUM partition j (other partitions accumulate zero). 256 accumulating
    bf16 matmuls build a full [128 q, 512 k] scores tile in one PSUM bank.
  - softmax over free dim: ACT Exp (PSUM src) with accum_out row sums,
    DVE reciprocal + tensor_scalar_mul -> attn f32 -> DMA out.
  - out = attn @ v: PE transpose of attn blocks -> bf16 attnT, bf16 v.
"""

import os

import numpy as np

B, QN, KVN = 4, 512, 512
QD, KD, H, VD = 512, 512, 256, 256
NCORES = 8
QSH = QN // 2  # 256 query rows per core
P = 128
QB = 16  # queries per tanh batch

_last_results = None


def _ensure_ntff_hook():
    """Register the NTFF profile hook so trace=True works under axon.

    The agent image's antenv package lacks axon_hooks, so trn_boot's
    silent-degrade path left concourse without a hook. Inject an
    in-memory module and install the ctypes hook from trn_agent_boot.
    """
    import sys
    import types

    try:
        import antenv.axon_hooks  # noqa: F401

        return
    except ImportError:
        pass
    try:
        import antenv
    except ImportError:
        return
    mod = types.ModuleType("antenv.axon_hooks")
    mod._hook = None

    def set_axon_ntff_profile_hook(hook):
        mod._hook = hook

    def get_axon_ntff_profile_hook():
        return mod._hook

    mod.set_axon_ntff_profile_hook = set_axon_ntff_profile_hook
    mod.get_axon_ntff_profile_hook = get_axon_ntff_profile_hook
    sys.modules["antenv.axon_hooks"] = mod
    antenv.axon_hooks = mod
    try:
        from trn_agent_boot.trn_boot import _ntff_profile_via_ctypes

        hook = _ntff_profile_via_ctypes("/opt/axon/libaxon_pjrt.so")
        if hook is not None:
            mod._hook = hook
    except Exception:
        pass


def _build_bass():
    import concourse.bass as bass  # noqa: F401
    import concourse.mybir as mybir
    import concourse.tile as tile
    from concourse import bacc
    from concourse.masks import make_identity

    f32 = mybir.dt.float32
    bf16 = mybir.dt.bfloat16
    AF = mybir.ActivationFunctionType

    nc = bacc.Bacc()

    q_d = nc.declare_dram_parameter("q", [QSH, QD], f32, isOutput=False)
    k_d = nc.declare_dram_parameter("k", [KVN, KD], f32, isOutput=False)
    v_d = nc.declare_dram_parameter("v", [KVN, VD], f32, isOutput=False)
    wq_d = nc.declare_dram_parameter("Wq", [H, QD], f32, isOutput=False)
    wk_d = nc.declare_dram_parameter("Wk", [H, KD], f32, isOutput=False)
    wv_d = nc.declare_dram_parameter("wv", [H], f32, isOutput=False)
    oy_d = nc.declare_dram_parameter("out_y", [QSH, VD], f32, isOutput=True)
    oa_d = nc.declare_dram_parameter("out_a", [QSH, KVN], f32, isOutput=True)

    DC = QD // P  # 4 contraction chunks
    HC = H // P  # 2 h chunks
    KC = KVN // P  # 4 kv chunks
    QT = QSH // P  # 2 query tiles per core

    with tile.TileContext(nc) as tc:
        with (
            tc.tile_pool(name="const", bufs=1) as const,
            tc.tile_pool(name="work", bufs=2) as work,
            tc.tile_pool(name="spool", bufs=2) as spool,
            tc.tile_pool(name="pre", bufs=6) as pre,
            tc.tile_pool(name="psum", bufs=1, space="PSUM") as psum,
        ):
            # ---- constants ----
            ident = const.tile([P, P], f32)
            make_identity(nc, ident)

            wv_pad = const.tile([P, HC, 2 * P - 1], bf16)
            nc.vector.memset(wv_pad, 0.0)
            wv_f32 = const.tile([P, HC], f32)
            for c in range(HC):
                nc.sync.dma_start(
                    out=wv_f32[:, c : c + 1], in_=wv_d[c * P : (c + 1) * P]
                )
                nc.vector.tensor_copy(wv_pad[:, c, P - 1 : P], wv_f32[:, c : c + 1])

            # ---- transposed loads: xT [d(part), seq] bf16, via PE transpose
            kT = const.tile([P, DC, KVN], bf16)
            qT = const.tile([P, DC, QSH], bf16)
            wqT = const.tile([P, DC, H], bf16)
            wkT = const.tile([P, DC, H], bf16)

            def load_transposed(dram, dst, dst_col0):
                nat = pre.tile([P, QD], f32, tag="nat")
                nc.sync.dma_start(out=nat, in_=dram)
                for dc in range(DC):
                    pt = psum.tile([P, P], f32, tag="pre_t", bufs=3)
                    nc.tensor.transpose(pt, nat[:, dc * P : (dc + 1) * P], ident)
                    nc.any.tensor_copy(
                        out=dst[:, dc, dst_col0 : dst_col0 + P], in_=pt
                    )

            # order: Wk, k first (kh feeds the first adds), then Wq, q
            for i in range(HC):
                load_transposed(wk_d[i * P : (i + 1) * P, :], wkT, i * P)
            for i in range(KC):
                load_transposed(k_d[i * P : (i + 1) * P, :], kT, i * P)
            for i in range(HC):
                load_transposed(wq_d[i * P : (i + 1) * P, :], wqT, i * P)
            for i in range(QT):
                load_transposed(q_d[i * P : (i + 1) * P, :], qT, i * P)

            # ---- kh [h, k] then qh [h, q], both bf16 ----
            kh = const.tile([P, HC, KVN], bf16)
            qh = const.tile([P, HC, QSH], f32)
            for c in range(HC):
                pk = psum.tile([P, KVN], f32, tag="sc", bufs=2)
                for dc in range(DC):
                    nc.tensor.matmul(
                        pk,
                        wkT[:, dc, c * P : (c + 1) * P],
                        kT[:, dc, :],
                        start=(dc == 0),
                        stop=(dc == DC - 1),
                    )
                nc.any.tensor_copy(out=kh[:, c, :], in_=pk)
            for c in range(HC):
                pq = psum.tile([P, QSH], f32, tag="sc", bufs=2)
                for dc in range(DC):
                    nc.tensor.matmul(
                        pq,
                        wqT[:, dc, c * P : (c + 1) * P],
                        qT[:, dc, :],
                        start=(dc == 0),
                        stop=(dc == DC - 1),
                    )
                nc.any.tensor_copy(out=qh[:, c, :], in_=pq)

            # ---- main loop over query tiles ----
            NB = P // QB  # blocks per query tile
            v_sb = const.tile([P, KC, VD], bf16)
            for qt in range(QT):
                psc = psum.tile([P, KVN], f32, tag="sc", bufs=2)
                for qb in range(NB):
                    S = spool.tile([P, HC, QB, KVN], bf16, tag="S")
                    E = spool.tile([P, HC, QB, KVN], bf16, tag="E")
                    # all c=0 adds first so the c=0 tanh can start early
                    for c in range(HC):
                        for j in range(QB):
                            qg = qt * P + qb * QB + j
                            nc.vector.tensor_scalar_add(
                                S[:, c, j, :], kh[:, c, :], qh[:, c, qg : qg + 1]
                            )
                    # split the first tanh batch (head latency) and the very
                    # last one (tail latency: shortens the final matvec burst)
                    first = qt == 0 and qb == 0
                    last = qt == QT - 1 and qb == NB - 1
                    nsub = 2 if first else (4 if last else 1)
                    sub = QB // nsub
                    for c in range(HC):
                        for s in range(nsub):
                            nc.scalar.activation(
                                E[:, c, s * sub : (s + 1) * sub],
                                S[:, c, s * sub : (s + 1) * sub],
                                AF.Tanh,
                            )
                    for j in range(QB):
                        ql = qb * QB + j
                        for c in range(HC):
                            nc.tensor.matmul(
                                psc,
                                wv_pad[:, c, P - 1 - ql : 2 * P - 1 - ql],
                                E[:, c, j, :],
                                start=(qb == 0 and j == 0 and c == 0),
                                stop=(qb == NB - 1 and j == QB - 1 and c == HC - 1),
                            )

                if qt == 0:
                    # v in bf16 [k, vd]: loaded here so its DMA stays off the
                    # critical preproc path (first needed by qt0's out matmul)
                    for kc in range(KC):
                        vnat = pre.tile([P, VD], f32, tag="vnat", bufs=2)
                        nc.sync.dma_start(
                            out=vnat, in_=v_d[kc * P : (kc + 1) * P, :]
                        )
                        nc.vector.tensor_copy(v_sb[:, kc, :], vnat)

                # softmax over k (free dim); no max subtraction needed
                # (|scores| <= ||wv||_1 ~ 13, exp stays in f32 range)
                probs = work.tile([P, KVN], f32, tag="probs")
                sums = work.tile([P, 1], f32, tag="sums")
                nc.scalar.activation(probs, psc, AF.Exp, accum_out=sums)
                rinv = work.tile([P, 1], f32, tag="rinv")
                nc.vector.reciprocal(rinv, sums)
                attn = work.tile([P, KVN], f32, tag="attn")
                nc.vector.tensor_scalar_mul(attn, probs, rinv)
                nc.sync.dma_start(out=oa_d[qt * P : (qt + 1) * P, :], in_=attn)

                # out = attn @ v : transpose attn blocks, accumulate over k
                po = psum.tile([P, VD], f32, tag="po", bufs=1)
                for kc in range(KC):
                    ptr = psum.tile([P, P], f32, tag="attn_t", bufs=2)
                    nc.tensor.transpose(
                        ptr, attn[:, kc * P : (kc + 1) * P], ident
                    )
                    attnT = work.tile([P, P], bf16, tag="attnT")
                    nc.vector.tensor_copy(attnT, ptr)
                    nc.tensor.matmul(
                        po,
                        attnT,
                        v_sb[:, kc, :],
                        start=(kc == 0),
                        stop=(kc == KC - 1),
                    )
                out_sb = work.tile([P, VD], f32, tag="out_sb")
                nc.vector.tensor_copy(out_sb, po)
                nc.sync.dma_start(out=oy_d[qt * P : (qt + 1) * P, :], in_=out_sb)

    nc.finalize()
    return nc


def kernel(q, k, v, Wq, Wk, wv):
    global _last_results
    _ensure_ntff_hook()
    from concourse.bass_utils import run_bass_kernel_spmd

    q = np.ascontiguousarray(np.asarray(q, dtype=np.float32))
    k = np.ascontiguousarray(np.asarray(k, dtype=np.float32))
    v = np.ascontiguousarray(np.asarray(v, dtype=np.float32))
    Wq = np.ascontiguousarray(np.asarray(Wq, dtype=np.float32))
    Wk = np.ascontiguousarray(np.asarray(Wk, dtype=np.float32))
    wv = np.ascontiguousarray(np.asarray(wv, dtype=np.float32))

    nc = _build_bass()

    in_maps = []
    for c in range(NCORES):
        b, half = c // 2, c % 2
        in_maps.append(
            {
                "q": np.ascontiguousarray(q[b, half * QSH : (half + 1) * QSH]),
                "k": k[b],
                "v": v[b],
                "Wq": Wq,
                "Wk": Wk,
                "wv": wv,
            }
        )

    trace = os.environ.get("KERNEL_TRACE", "0") == "1"
    res = run_bass_kernel_spmd(
        nc, in_maps, core_ids=list(range(NCORES)), trace=trace
    )
    _last_results = res

    output = np.empty((B, QN, VD), dtype=np.float32)
    attention = np.empty((B, QN, KVN), dtype=np.float32)
    for c in range(NCORES):
        b, half = c // 2, c % 2
        output[b, half * QSH : (half + 1) * QSH] = res.results[c]["out_y"]
        attention[b, half * QSH : (half + 1) * QSH] = res.results[c]["out_a"]
    return output, attention


# revision 18
# speedup vs baseline: 1.1279x; 1.0152x over previous
"""AdditiveAttention kernel for 8 TRN2 NeuronCores.

Problem shapes (hardcoded): q [4,512,512], k [4,512,512], v [4,512,256],
Wq [256,512], Wk [256,512], wv [256].

reference:
    qh = q @ Wq.T            [B, QN, H]
    kh = k @ Wk.T            [B, KVN, H]
    scores[b,q,k] = sum_h wv[h] * tanh(qh[b,q,h] + kh[b,k,h])
    attn = softmax(scores, axis=-1)
    out = attn @ v
    returns (out, attn)

Sharding: core c handles batch b = c//2, query rows (c%2)*256:(c%2+1)*256.
k, v, Wq, Wk, wv replicated per batch. No collectives.

Per-core kernel strategy (ACT/tanh-roofline bound, ~33.5M tanh elems/core):
  - qh, kh computed on-chip in [h(part), seq(free)] layout (PE transposes
    of q/k/W via identity into bf16, then bf16 matmuls).
  - energy S = qh[:,q] + kh via DVE tensor_scalar_add (per-partition
    scalar, bf16 in/out -> 4x DVE mode), batched QB queries per buffer;
    tanh in one big ACT instruction per h-chunk (bf16 -> bf16).
  - scores row for query q via PE matvec with a sliding-window one-hot
    stationary: wv_pad [128, 2, 255] zeros with wv chunk at col 127; lhsT
    window [:, c, 127-j:255-j] puts wv in column j so the matvec writes
    PSUM partition j (other partitions accumulate zero). 256 accumulating
    bf16 matmuls build a full [128 q, 512 k] scores tile in one PSUM bank.
  - softmax over free dim: ACT Exp (PSUM src) with accum_out row sums,
    DVE reciprocal + tensor_scalar_mul -> attn f32 -> DMA out.
  - out = attn @ v: PE transpose of attn blocks -> bf16 attnT, bf16 v.
"""

import os

import numpy as np

B, QN, KVN = 4, 512, 512
QD, KD, H, VD = 512, 512, 256, 256
NCORES = 8
QSH = QN // 2  # 256 query rows per core
P = 128
QB = 16  # queries per tanh batch

_last_results = None


def _ensure_ntff_hook():
    """Register the NTFF profile hook so trace=True works under axon.

    The agent image's antenv package lacks axon_hooks, so trn_boot's
    silent-degrade path left concourse without a hook. Inject an
    in-memory module and install the ctypes hook from trn_agent_boot.
    """
    import sys
    import types

    try:
        import antenv.axon_hooks  # noqa: F401

        return
    except ImportError:
        pass
    try:
        import antenv
    except ImportError:
        return
    mod = types.ModuleType("antenv.axon_hooks")
    mod._hook = None

    def set_axon_ntff_profile_hook(hook):
        mod._hook = hook

    def get_axon_ntff_profile_hook():
        return mod._hook

    mod.set_axon_ntff_profile_hook = set_axon_ntff_profile_hook
    mod.get_axon_ntff_profile_hook = get_axon_ntff_profile_hook
    sys.modules["antenv.axon_hooks"] = mod
    antenv.axon_hooks = mod
    try:
        from trn_agent_boot.trn_boot import _ntff_profile_via_ctypes

        hook = _ntff_profile_via_ctypes("/opt/axon/libaxon_pjrt.so")
        if hook is not None:
            mod._hook = hook
    except Exception:
        pass


def _build_bass():
    import concourse.bass as bass  # noqa: F401
    import concourse.mybir as mybir
    import concourse.tile as tile
    from concourse import bacc
    from concourse.masks import make_identity

    f32 = mybir.dt.float32
    bf16 = mybir.dt.bfloat16
    AF = mybir.ActivationFunctionType

    nc = bacc.Bacc()

    q_d = nc.declare_dram_parameter("q", [QSH, QD], f32, isOutput=False)
    k_d = nc.declare_dram_parameter("k", [KVN, KD], f32, isOutput=False)
    v_d = nc.declare_dram_parameter("v", [KVN, VD], f32, isOutput=False)
    wq_d = nc.declare_dram_parameter("Wq", [H, QD], f32, isOutput=False)
    wk_d = nc.declare_dram_parameter("Wk", [H, KD], f32, isOutput=False)
    wv_d = nc.declare_dram_parameter("wv", [H], f32, isOutput=False)
    oy_d = nc.declare_dram_parameter("out_y", [QSH, VD], f32, isOutput=True)
    oa_d = nc.declare_dram_parameter("out_a", [QSH, KVN], f32, isOutput=True)

    DC = QD // P  # 4 contraction chunks
    HC = H // P  # 2 h chunks
    KC = KVN // P  # 4 kv chunks
    QT = QSH // P  # 2 query tiles per core

    with tile.TileContext(nc) as tc:
        with (
            tc.tile_pool(name="const", bufs=1) as const,
            tc.tile_pool(name="work", bufs=2) as work,
            tc.tile_pool(name="spool", bufs=2) as spool,
            tc.tile_pool(name="pre", bufs=6) as pre,
            tc.tile_pool(name="psum", bufs=1, space="PSUM") as psum,
        ):
            # ---- constants ----
            ident = const.tile([P, P], bf16)
            make_identity(nc, ident)
            ident_f = const.tile([P, P], f32)
            make_identity(nc, ident_f)

            # ---- transposed loads: xT [d(part), seq] bf16, via bf16 PE
            # transpose (cast f32 nat -> bf16 first: faster LDW + matmul)
            kT = const.tile([P, DC, KVN], bf16)
            qT = const.tile([P, DC, QSH], bf16)
            wqT = const.tile([P, DC, H], bf16)
            wkT = const.tile([P, DC, H], bf16)

            def load_transposed(dram, dst, dst_col0):
                nat = pre.tile([P, QD], f32, tag="nat")
                nc.sync.dma_start(out=nat, in_=dram)
                natb = pre.tile([P, QD], bf16, tag="natb")
                nc.vector.tensor_copy(natb, nat)
                for dc in range(DC):
                    pt = psum.tile([P, P], bf16, tag="pre_t", bufs=3)
                    nc.tensor.transpose(pt, natb[:, dc * P : (dc + 1) * P], ident)
                    nc.any.tensor_copy(
                        out=dst[:, dc, dst_col0 : dst_col0 + P], in_=pt
                    )

            # order: Wk, k first (kh feeds the first adds), then Wq, q
            for i in range(HC):
                load_transposed(wk_d[i * P : (i + 1) * P, :], wkT, i * P)
            for i in range(KC):
                load_transposed(k_d[i * P : (i + 1) * P, :], kT, i * P)
            for i in range(HC):
                load_transposed(wq_d[i * P : (i + 1) * P, :], wqT, i * P)
            for i in range(QT):
                load_transposed(q_d[i * P : (i + 1) * P, :], qT, i * P)

            wv_pad = const.tile([P, HC, 2 * P - 1], bf16)
            nc.vector.memset(wv_pad, 0.0)
            wv_f32 = const.tile([P, HC], f32)
            for c in range(HC):
                nc.sync.dma_start(
                    out=wv_f32[:, c : c + 1], in_=wv_d[c * P : (c + 1) * P]
                )
                nc.vector.tensor_copy(wv_pad[:, c, P - 1 : P], wv_f32[:, c : c + 1])

            # ---- kh [h, k] then qh [h, q], both bf16 ----
            kh = const.tile([P, HC, KVN], bf16)
            qh = const.tile([P, HC, QSH], f32)
            for c in range(HC):
                pk = psum.tile([P, KVN], f32, tag="sc", bufs=2)
                for dc in range(DC):
                    nc.tensor.matmul(
                        pk,
                        wkT[:, dc, c * P : (c + 1) * P],
                        kT[:, dc, :],
                        start=(dc == 0),
                        stop=(dc == DC - 1),
                    )
                nc.any.tensor_copy(out=kh[:, c, :], in_=pk)
            for c in range(HC):
                pq = psum.tile([P, QSH], f32, tag="sc", bufs=2)
                for dc in range(DC):
                    nc.tensor.matmul(
                        pq,
                        wqT[:, dc, c * P : (c + 1) * P],
                        qT[:, dc, :],
                        start=(dc == 0),
                        stop=(dc == DC - 1),
                    )
                nc.any.tensor_copy(out=qh[:, c, :], in_=pq)

            # ---- main loop over query tiles ----
            NB = P // QB  # blocks per query tile
            v_sb = const.tile([P, KC, VD], bf16)
            for qt in range(QT):
                psc = psum.tile([P, KVN], f32, tag="sc", bufs=2)
                for qb in range(NB):
                    S = spool.tile([P, HC, QB, KVN], bf16, tag="S")
                    E = spool.tile([P, HC, QB, KVN], bf16, tag="E")
                    # all c=0 adds first so the c=0 tanh can start early
                    for c in range(HC):
                        for j in range(QB):
                            qg = qt * P + qb * QB + j
                            nc.vector.tensor_scalar_add(
                                S[:, c, j, :], kh[:, c, :], qh[:, c, qg : qg + 1]
                            )
                    # split the first tanh batch (head latency) and the very
                    # last one (tail latency: shortens the final matvec burst)
                    first = qt == 0 and qb == 0
                    last = qt == QT - 1 and qb == NB - 1
                    if first or last:
                        nsub = 2 if first else 4
                        sub = QB // nsub
                        for c in range(HC):
                            for s in range(nsub):
                                nc.scalar.activation(
                                    E[:, c, s * sub : (s + 1) * sub],
                                    S[:, c, s * sub : (s + 1) * sub],
                                    AF.Tanh,
                                )
                    else:
                        # both h-chunks in one big ACT instruction
                        nc.scalar.activation(E, S, AF.Tanh)
                    for j in range(QB):
                        ql = qb * QB + j
                        for c in range(HC):
                            nc.tensor.matmul(
                                psc,
                                wv_pad[:, c, P - 1 - ql : 2 * P - 1 - ql],
                                E[:, c, j, :],
                                start=(qb == 0 and j == 0 and c == 0),
                                stop=(qb == NB - 1 and j == QB - 1 and c == HC - 1),
                            )

                if qt == 0:
                    # v in bf16 [k, vd]: loaded here so its DMA stays off the
                    # critical preproc path (first needed by qt0's out matmul)
                    for kc in range(KC):
                        vnat = pre.tile([P, VD], f32, tag="vnat", bufs=2)
                        nc.sync.dma_start(
                            out=vnat, in_=v_d[kc * P : (kc + 1) * P, :]
                        )
                        nc.vector.tensor_copy(v_sb[:, kc, :], vnat)

                # softmax over k (free dim); no max subtraction needed
                # (|scores| <= ||wv||_1 ~ 13, exp stays in f32 range)
                probs = work.tile([P, KVN], f32, tag="probs")
                sums = work.tile([P, 1], f32, tag="sums")
                nc.scalar.activation(probs, psc, AF.Exp, accum_out=sums)
                rinv = work.tile([P, 1], f32, tag="rinv")
                nc.vector.reciprocal(rinv, sums)
                attn = work.tile([P, KVN], f32, tag="attn")
                nc.vector.tensor_scalar_mul(attn, probs, rinv)
                nc.sync.dma_start(out=oa_d[qt * P : (qt + 1) * P, :], in_=attn)

                # out = attn @ v : transpose attn blocks, accumulate over k
                po = psum.tile([P, VD], f32, tag="po", bufs=1)
                for kc in range(KC):
                    ptr = psum.tile([P, P], f32, tag="attn_t", bufs=2)
                    nc.tensor.transpose(
                        ptr, attn[:, kc * P : (kc + 1) * P], ident_f
                    )
                    attnT = work.tile([P, P], bf16, tag="attnT")
                    nc.vector.tensor_copy(attnT, ptr)
                    nc.tensor.matmul(
                        po,
                        attnT,
                        v_sb[:, kc, :],
                        start=(kc == 0),
                        stop=(kc == KC - 1),
                    )
                out_sb = work.tile([P, VD], f32, tag="out_sb")
                nc.vector.tensor_copy(out_sb, po)
                nc.sync.dma_start(out=oy_d[qt * P : (qt + 1) * P, :], in_=out_sb)

    nc.finalize()
    return nc


def kernel(q, k, v, Wq, Wk, wv):
    global _last_results
    _ensure_ntff_hook()
    from concourse.bass_utils import run_bass_kernel_spmd

    q = np.ascontiguousarray(np.asarray(q, dtype=np.float32))
    k = np.ascontiguousarray(np.asarray(k, dtype=np.float32))
    v = np.ascontiguousarray(np.asarray(v, dtype=np.float32))
    Wq = np.ascontiguousarray(np.asarray(Wq, dtype=np.float32))
    Wk = np.ascontiguousarray(np.asarray(Wk, dtype=np.float32))
    wv = np.ascontiguousarray(np.asarray(wv, dtype=np.float32))

    nc = _build_bass()

    in_maps = []
    for c in range(NCORES):
        b, half = c // 2, c % 2
        in_maps.append(
            {
                "q": np.ascontiguousarray(q[b, half * QSH : (half + 1) * QSH]),
                "k": k[b],
                "v": v[b],
                "Wq": Wq,
                "Wk": Wk,
                "wv": wv,
            }
        )

    trace = os.environ.get("KERNEL_TRACE", "0") == "1"
    res = run_bass_kernel_spmd(
        nc, in_maps, core_ids=list(range(NCORES)), trace=trace
    )
    _last_results = res

    output = np.empty((B, QN, VD), dtype=np.float32)
    attention = np.empty((B, QN, KVN), dtype=np.float32)
    for c in range(NCORES):
        b, half = c // 2, c % 2
        output[b, half * QSH : (half + 1) * QSH] = res.results[c]["out_y"]
        attention[b, half * QSH : (half + 1) * QSH] = res.results[c]["out_a"]
    return output, attention
